# revision 8
# baseline (speedup 1.0000x reference)
"""Trainium2 Bass kernel for nn_BackboneModule (AlphaFold-style build_structure).

Strategy: pure data-parallel over 8 cores. Within each core, residues are
packed HOST-side into a [126, L] grid sorted by residue type: partition row
p = 6*tau + k holds only residues of type tau.  Every type-dependent constant
(transforms table, rigids table, dependency-tree selection masks) therefore
becomes a per-partition [126,1] scalar, so the whole kinematic chain +
atom placement is expressed with fused scalar_tensor_tensor FMAs and
per-partition-scalar ops.  No data-dependent gather runs on device.
"""

import numpy as np

import concourse.bass as bass
import concourse.mybir as mybir
from concourse.tile import TileContext
from concourse.bass_utils import run_bass_kernel_spmd

F32 = mybir.dt.float32

N_RES = 300000
NT = 21          # residue types
NR = 8           # MAX_RIGID
NA = 24          # MAX_ATOM
NCORES = 8
RES_PER_CORE = N_RES // NCORES   # 37500

P = 126          # partitions used = 21 types * 6 rows
RPT = 6          # partition rows per type
CL = 128         # columns per chunk
NCHUNK = 3
L = CL * NCHUNK  # 384 columns; capacity 126*384 = 48384 >= 37500 (+pad)

# const column layout
def ct(r, i, j):  # transforms[r][i][j], i in 0..3 (row 3 = translation)
    return r * 12 + i * 3 + j
def rg(a, j):     # rigids[a][j]
    return 96 + a * 3 + j
_CM_OFF = 96 + 72
_cm_idx = {}
_off = _CM_OFF
for _i in range(2, NR):
    for _j in range(_i):
        _cm_idx[(_i, _j)] = _off
        _off += 1
_AM_OFF = _off                 # 195
NCOLS = _AM_OFF + NA * NR      # 195 + 192 = 387
def cm(i, j):
    return _cm_idx[(i, j)]
def am(a, s):
    return _AM_OFF + a * NR + s

TRACE = False
_RESULT_CACHE = {}


def _build_program(nc):
    inp = nc.dram_tensor("inp", [P, 24, L], F32, kind="ExternalInput")
    cst = nc.dram_tensor("cst", [P, NCOLS], F32, kind="ExternalInput")
    outR = nc.dram_tensor("outR", [P, NA * 3, L], F32, kind="ExternalOutput")
    outO = nc.dram_tensor("outO", [P, 12, L], F32, kind="ExternalOutput")

    with nc.sbuf_tensor([P, 24, CL], F32) as tin, \
         nc.sbuf_tensor([P, NCOLS], F32) as tcst, \
         nc.sbuf_tensor([P, NA * 3, CL], F32) as tR, \
         nc.sbuf_tensor([P, 12 * NR, CL], F32) as toprs, \
         nc.sbuf_tensor([P, 12, CL], F32) as A, \
         nc.sbuf_tensor([P, 12, CL], F32) as B, \
         nc.sbuf_tensor([P, 12, CL], F32) as Rg, \
         nc.sbuf_tensor([P, 16, CL], F32) as tq, \
         nc.sbuf_tensor([P, 12, CL], F32) as T12, \
         nc.sbuf_tensor([P, 3, CL], F32) as acc, \
         nc.semaphore() as dsem, \
         nc.semaphore() as vsem, \
         nc.semaphore() as osem, \
         nc.Block() as block:

        opr = [toprs[:, 12 * s : 12 * (s + 1), :] for s in range(NR)]

        @block.sync
        def _(sync):
            sync.dma_start(tcst[:], cst[:]).then_inc(dsem, 16)
            for ch in range(NCHUNK):
                if ch > 0:
                    sync.wait_ge(vsem, ch)       # tin free (compute ch-1 done)
                    sync.wait_ge(osem, 16 * ch)  # tR free (out-DMA ch-1 done)
                sync.dma_start(
                    tin[:], inp[:, :, ch * CL : (ch + 1) * CL]
                ).then_inc(dsem, 16)
                sync.wait_ge(vsem, ch + 1)       # compute ch done
                sync.dma_start(
                    outR[:, :, ch * CL : (ch + 1) * CL], tR[:]
                ).then_inc(osem, 16)
                sync.dma_start(
                    outO[:, :, ch * CL : (ch + 1) * CL], opr[0]
                ).then_inc(osem, 16)

        @block.vector
        def _(v):
            def C(k):
                return tcst[:, k : k + 1]
            r0, r1, r2 = acc[:, 0, :], acc[:, 1, :], acc[:, 2, :]
            MU, AD = mybir.AluOpType.mult, mybir.AluOpType.add

            for ch in range(NCHUNK):
                v.wait_ge(dsem, 16 * (ch + 2))   # cst + this chunk's tin landed
                if ch > 0:
                    # opr[0] (= outO src) rewrite hazard vs out-DMA of ch-1
                    v.wait_ge(osem, 32 * ch)

                def IN(k):
                    return tin[:, k, :]
                w, x, y, z = IN(0), IN(1), IN(2), IN(3)

                # ---- quaternion -> R0 (tq rows 0..8), t0 (rows 9..11)
                ww, xx, yy, zz = tq[:, 12, :], tq[:, 13, :], tq[:, 14, :], tq[:, 15, :]
                v.tensor_mul(ww, w, w)
                v.tensor_mul(xx, x, x)
                v.tensor_mul(yy, y, y)
                v.tensor_mul(zz, z, z)
                v.tensor_add(r0, ww, xx)
                v.tensor_add(r1, yy, zz)
                v.tensor_sub(tq[:, 0, :], r0, r1)          # r00
                v.tensor_sub(r0, ww, xx)
                v.tensor_sub(r1, yy, zz)
                v.tensor_add(tq[:, 4, :], r0, r1)          # r11
                v.tensor_sub(tq[:, 8, :], r0, r1)          # r22
                def cross2(dst, u1, u2):
                    v.tensor_scalar_mul(r0, u1, 2.0)
                    v.tensor_mul(dst, r0, u2)
                xy2, xz2, yz2 = ww, xx, yy  # rows 12,13,14 (diag done)
                wz2 = zz                                    # row 15
                cross2(xy2, x, y)
                cross2(xz2, x, z)
                cross2(yz2, y, z)
                cross2(wz2, w, z)
                v.tensor_sub(tq[:, 1, :], xy2, wz2)        # r01
                v.tensor_add(tq[:, 3, :], xy2, wz2)        # r10
                wy2 = tq[:, 12, :]                          # xy2 done
                cross2(wy2, w, y)
                v.tensor_add(tq[:, 2, :], xz2, wy2)        # r02
                v.tensor_sub(tq[:, 6, :], xz2, wy2)        # r20
                wx2 = tq[:, 13, :]                          # xz2 done
                cross2(wx2, w, x)
                v.tensor_sub(tq[:, 5, :], yz2, wx2)        # r12
                v.tensor_add(tq[:, 7, :], yz2, wx2)        # r21
                for j in range(3):                          # t0 = 0.1*t + pos0
                    v.tensor_scalar_mul(r0, IN(4 + j), 0.1)
                    v.tensor_add(tq[:, 9 + j, :], r0, IN(7 + j))

                def dot3(dst, s0, s1, s2, c0, c1, c2, cadd=None):
                    v.tensor_scalar_mul(r0, s0, C(c0))
                    v.tensor_scalar_mul(r1, s1, C(c1))
                    v.tensor_add(r2, r0, r1)
                    v.tensor_scalar_mul(r0, s2, C(c2))
                    if cadd is None:
                        v.tensor_add(dst, r2, r0)
                    else:
                        v.tensor_add(r1, r2, r0)
                        v.tensor_scalar_add(dst, r1, C(cadd))

                def combine_const_left(dst, src, r):
                    for i in range(3):
                        for k in range(3):
                            dot3(dst[:, i * 3 + k, :], src[:, k, :], src[:, 3 + k, :],
                                 src[:, 6 + k, :], ct(r, i, 0), ct(r, i, 1), ct(r, i, 2))
                        dot3(dst[:, 9 + i, :], src[:, 9, :], src[:, 10, :], src[:, 11, :],
                             ct(r, i, 0), ct(r, i, 1), ct(r, i, 2), cadd=ct(r, 3, i))

                # dst = src o T_r : R = src_R @ T_R ; t = src_R @ T_t + src_t
                def combine_const_right(dst, src, r):
                    for i in range(3):
                        for k in range(3):
                            dot3(dst[:, i * 3 + k, :], src[:, i * 3, :],
                                 src[:, i * 3 + 1, :], src[:, i * 3 + 2, :],
                                 ct(r, 0, k), ct(r, 1, k), ct(r, 2, k))
                        v.tensor_scalar_mul(r0, src[:, i * 3, :], C(ct(r, 3, 0)))
                        v.tensor_scalar_mul(r1, src[:, i * 3 + 1, :], C(ct(r, 3, 1)))
                        v.tensor_add(r2, r0, r1)
                        v.tensor_scalar_mul(r0, src[:, i * 3 + 2, :], C(ct(r, 3, 2)))
                        v.tensor_add(r1, r2, r0)
                        v.tensor_add(dst[:, 9 + i, :], r1, src[:, 9 + i, :])

                combine_const_left(opr[0], tq, 0)

                def macc(oth, j, col, cur):
                    v.tensor_scalar_mul(T12[:], opr[j], C(col))
                    v.tensor_add(oth[:] if hasattr(oth, "ap") is False else oth, T12[:], cur)

                for i in range(1, NR):
                    if i == 1:
                        Asrc = opr[0]
                    else:
                        v.tensor_scalar_mul(A[:], opr[0], C(cm(i, 0)))
                        cur, oth = A[:], Rg[:]
                        for j in range(1, i):
                            v.tensor_scalar_mul(T12[:], opr[j], C(cm(i, j)))
                            v.tensor_add(oth, T12[:], cur)
                            cur, oth = oth, cur
                        Asrc = cur
                    combine_const_right(B, Asrc, i)
                    ci, si = IN(10 + i - 1), IN(17 + i - 1)
                    for j in range(3):
                        v.tensor_copy(opr[i][:, j * 3 + 0, :], B[:, j * 3 + 0, :])
                        v.tensor_copy(opr[i][:, 9 + j, :], B[:, 9 + j, :])
                        v.tensor_mul(r0, B[:, j * 3 + 1, :], ci)
                        v.tensor_mul(r1, B[:, j * 3 + 2, :], si)
                        v.tensor_add(opr[i][:, j * 3 + 1, :], r0, r1)
                        v.tensor_mul(r0, B[:, j * 3 + 2, :], ci)
                        v.tensor_mul(r1, B[:, j * 3 + 1, :], si)
                        v.tensor_sub(opr[i][:, j * 3 + 2, :], r0, r1)

                for a in range(NA):
                    v.tensor_scalar_mul(Rg[:], opr[0], C(am(a, 0)))
                    cur, oth = Rg[:], A[:]
                    for s in range(1, NR):
                        v.tensor_scalar_mul(T12[:], opr[s], C(am(a, s)))
                        v.tensor_add(oth, T12[:], cur)
                        cur, oth = oth, cur
                    G = cur
                    last = None
                    for i in range(3):
                        dot3(r2, G[:, i * 3, :], G[:, i * 3 + 1, :], G[:, i * 3 + 2, :],
                             rg(a, 0), rg(a, 1), rg(a, 2))
                        last = v.tensor_add(tR[:, a * 3 + i, :], r2, G[:, 9 + i, :])
                if True:
                    last.then_inc(vsem, 1)
    return nc


def kernel(bb, sc, pos0, transforms_tensor, rigids_tensor,
           residue_type, transforms_dep, rigids_dep):
    bb = np.asarray(bb, dtype=np.float32)
    sc_in = np.asarray(sc, dtype=np.float32)
    pos0 = np.asarray(pos0, dtype=np.float32)
    tt = np.asarray(transforms_tensor, dtype=np.float32)
    rt_tab = np.asarray(rigids_tensor, dtype=np.float32)
    rtyp = np.asarray(residue_type).astype(np.int64)
    td = np.asarray(transforms_dep).astype(np.int64)
    rd = np.asarray(rigids_dep).astype(np.int64)

    # per-type constant rows [NT, NCOLS], replicated to [P, NCOLS]
    cst_t = np.zeros((NT, NCOLS), dtype=np.float32)
    for t in range(NT):
        cst_t[t, 0:96] = tt[t].reshape(-1)            # [8,4,3]
        cst_t[t, 96:168] = rt_tab[t].reshape(-1)      # [24,3]
        for i in range(2, NR):
            for j in range(i):
                cst_t[t, cm(i, j)] = 1.0 if td[t, i] == j else 0.0
        for a in range(NA):
            for s in range(NR):
                cst_t[t, am(a, s)] = 1.0 if rd[t, a] == s else 0.0
    CST = np.repeat(cst_t, RPT, axis=0)               # [126, NCOLS]

    in_maps = []
    placement = []   # (p_arr, c_arr) per core
    for c in range(NCORES):
        lo = c * RES_PER_CORE
        rt_l = rtyp[lo : lo + RES_PER_CORE]
        IN = np.zeros((P, 24, L), dtype=np.float32)
        p_arr = np.zeros(RES_PER_CORE, dtype=np.int64)
        c_arr = np.zeros(RES_PER_CORE, dtype=np.int64)
        comps = np.concatenate([
            bb[lo : lo + RES_PER_CORE, 0:4],                    # w x y z
            bb[lo : lo + RES_PER_CORE, 4:7],                    # t
            pos0[lo : lo + RES_PER_CORE],                       # pos0
            sc_in[lo : lo + RES_PER_CORE, :, 0],                # c1..c7
            sc_in[lo : lo + RES_PER_CORE, :, 1],                # s1..s7
        ], axis=1)                                              # [n, 24]
        for t in range(NT):
            idx = np.nonzero(rt_l == t)[0]
            cnt = len(idx)
            assert cnt <= RPT * L, f"type {t} count {cnt} exceeds capacity"
            jj = np.arange(cnt)
            rows = t * RPT + jj // L
            cols = jj % L
            p_arr[idx] = rows
            c_arr[idx] = cols
            IN[rows, :, cols] = comps[idx]
        in_maps.append({"inp": IN, "cst": CST})
        placement.append((p_arr, c_arr))

    nc = bass.Bass("TRN2", target_bir_lowering=False)
    _build_program(nc)
    res = run_bass_kernel_spmd(nc, in_maps, core_ids=list(range(NCORES)),
                               trace=TRACE)
    _RESULT_CACHE["last"] = res
    _RESULT_CACHE["nc"] = nc
    _RESULT_CACHE["in_maps"] = in_maps

    R_full = np.zeros((N_RES, NA, 3), dtype=np.float32)
    O_full = np.zeros((N_RES, 4, 3), dtype=np.float32)
    for c in range(NCORES):
        p_arr, c_arr = placement[c]
        lo = c * RES_PER_CORE
        Rdev = res.results[c]["outR"]                 # [P, 72, L]
        Odev = res.results[c]["outO"]                 # [P, 12, L]
        R_full[lo : lo + RES_PER_CORE] = Rdev[p_arr, :, c_arr].reshape(-1, NA, 3)
        O_full[lo : lo + RES_PER_CORE] = Odev[p_arr, :, c_arr].reshape(-1, 4, 3)
    return R_full, O_full


def bench(n_iter=5):
    """Time repeat PJRT executions of the prebuilt module (compile cached).
    Returns per-iteration wall seconds, incl. H2D input transfer."""
    import time
    import jax
    import jax.numpy as jnp
    from jax.sharding import Mesh, PartitionSpec
    from jax.experimental.shard_map import shard_map
    from concourse import bass2jax

    nc = _RESULT_CACHE["nc"]
    in_maps = _RESULT_CACHE["in_maps"]
    # one warm call through the normal path (jit+neff cache hot afterwards)
    ts = []
    for i in range(n_iter):
        t0 = time.time()
        bass2jax.run_bass_via_pjrt(nc, in_maps, n_cores=NCORES)
        ts.append(time.time() - t0)
    return ts
